# revision 11
# baseline (speedup 1.0000x reference)
"""CapsuleLayer Trainium2 kernel.

Math (per batch element), algebraically reformulated to avoid materializing
u_hat = x @ W.T ([S, 512], 4MB/batch) entirely:

  b_r[o, s]  = sum_i x[s, i] * G_r[o, i]          (G_0 = 0 -> c_0 uniform)
  c_r        = softmax_o(b_r)
  M_r[o, i]  = sum_s c_r[o, s] * x[s, i]          (iter 0: M_0 = Xsum / 16)
  s_r[o, d]  = sum_i W[(o,d), i] * M_r[o, i]
  a_r        = squash(s_r)
  G_{r+1}    = G_r + ArW,  ArW[o, i] = sum_d a_r[o, d] * W[(o,d), i]

Sharding: pure data parallel, batch 32 -> 8 cores x 4.
"""

import sys

sys.path.insert(0, "/opt/trn_rl_repo")

from contextlib import ExitStack

import numpy as np

import concourse.bass as bass
import concourse.bacc as bacc
import concourse.tile as tile
from concourse import masks, mybir
from concourse._compat import with_exitstack

F32 = mybir.dt.float32
AX = mybir.AxisListType
ALU = mybir.AluOpType
ACTF = mybir.ActivationFunctionType

B = 4  # batch elements per core
S = 2048
I = 128
O = 16
D = 32
NCH = S // 128  # 16 s-chunks
N_CORES = 8


@with_exitstack
def _build(ctx: ExitStack, tc: "tile.TileContext"):
    nc = tc.nc

    x_d = nc.dram_tensor("x", [B, S, I], F32, kind="ExternalInput")
    w_d = nc.dram_tensor("W", [O * D, I, 1], F32, kind="ExternalInput")
    y_d = nc.dram_tensor("y", [B, O, D], F32, kind="ExternalOutput")

    const = ctx.enter_context(tc.tile_pool(name="const", bufs=1))
    xpool = ctx.enter_context(tc.tile_pool(name="xpool", bufs=1))
    work = ctx.enter_context(tc.tile_pool(name="work", bufs=2))
    ps_big = ctx.enter_context(tc.tile_pool(name="ps_big", bufs=2, space="PSUM"))
    ps_mt = ctx.enter_context(tc.tile_pool(name="ps_mt", bufs=1, space="PSUM"))
    ps_sm = ctx.enter_context(tc.tile_pool(name="ps_sm", bufs=2, space="PSUM"))

    # ---- constants ----
    ident = const.tile([128, 128], F32, tag="ident")
    masks.make_identity(nc, ident[:])

    # W natural, chunked over od: W4[p, c, i] = W[128c + p, i]
    w4 = const.tile([128, 4, I], F32, tag="w4")
    nc.sync.dma_start(w4[:], w_d[:, :, 0].rearrange("(c p) i -> p c i", p=128))
    # W transposed: wt[i, od] = W[od, i]
    wt = const.tile([128, O * D], F32, tag="wt")
    for c in range(4):
        tp = ps_big.tile([128, 128], F32, tag="tp")
        nc.tensor.transpose(tp[:], w4[:, c, :], ident[:])
        nc.scalar.copy(wt[:, 128 * c : 128 * (c + 1)], tp[:])

    ones32 = const.tile([32, 1], F32, tag="ones32")
    nc.vector.memset(ones32[:], 1.0)
    ones1 = const.tile([1, 32], F32, tag="ones1")
    nc.vector.memset(ones1[:], 1.0)

    # p4t[d, j] = 1 if j % 32 == d  (partition replicate selector)
    p4t = const.tile([32, 128], F32, tag="p4t")
    nc.gpsimd.memset(p4t[:], 0.0)
    nc.gpsimd.affine_select(
        out=p4t[:].rearrange("p (g q) -> p g q", q=32),
        in_=p4t[:].rearrange("p (g q) -> p g q", q=32),
        pattern=[[0, 4], [-1, 32]],  # iota = d - q
        compare_op=ALU.not_equal,
        fill=1.0,
        base=0,
        channel_multiplier=1,
    )

    # amask[j, c, bo] = 1 if (bo % 16) == 4c + j // 32.
    # With v = j + 128c - 32*(bo % 16): condition <=> 0 <= v <= 31.
    amask = const.tile([128, 4, B * O], F32, tag="amask")
    nc.gpsimd.memset(amask[:], 1.0)
    amv = amask[:].rearrange("p c (b o) -> p c b o", o=O)
    nc.gpsimd.affine_select(
        out=amv, in_=amv,
        pattern=[[128, 4], [0, B], [-32, O]],
        compare_op=ALU.is_ge,  # keep where v >= 0
        fill=0.0, base=0, channel_multiplier=1,
    )
    nc.gpsimd.affine_select(
        out=amv, in_=amv,
        pattern=[[-128, 4], [0, B], [32, O]],
        compare_op=ALU.is_ge,  # keep where 31 - v >= 0
        fill=0.0, base=31, channel_multiplier=-1,
    )

    # ---- load x (per batch, per quarter) ----
    xnat = [
        [
            xpool.tile([128, 4, I], F32, tag=f"xn_{b}_{q}", name=f"xn_{b}_{q}")
            for q in range(4)
        ]
        for b in range(B)
    ]
    for b in range(B):
        for q in range(4):
            nc.sync.dma_start(
                xnat[b][q][:],
                x_d[b, 512 * q : 512 * (q + 1), :].rearrange("(c p) i -> p c i", p=128),
            )

    # ---- phase T: build xT[b][i, c, s'] = x[b, 128c + s', i]; fold in col-sums ----
    xT = [
        xpool.tile([128, NCH, 128], F32, tag=f"xT_{b}", name=f"xT_{b}")
        for b in range(B)
    ]
    acc = xpool.tile([128, B, NCH], F32, tag="acc")
    for b in range(B):
        for c in range(NCH):
            tp = ps_big.tile([128, 128], F32, tag="tp")
            nc.tensor.transpose(tp[:], xnat[b][c // 4][:, c % 4, :], ident[:])
            if (b * NCH + c) % 2 == 0:
                nc.scalar.activation(
                    xT[b][:, c, :], tp[:], ACTF.Copy, accum_out=acc[:, b, c : c + 1]
                )
            else:
                nc.vector.tensor_scalar(
                    xT[b][:, c, :], tp[:], 1.0, None, ALU.mult, ALU.add,
                    accum_out=acc[:, b, c : c + 1],
                )
    xsum = xpool.tile([128, B], F32, tag="xsum")  # xsum[i, b] = sum_s x[b, s, i]
    nc.vector.reduce_sum(xsum[:], acc[:], axis=AX.X)

    # ---- G^T accumulator lives in SBUF: gt[i, 16b + o] ----
    gt_sb = None

    for r in range(3):
        if r == 0:
            mt_sb = None
        else:
            # b_r and softmax -> cT, per batch
            cT = []
            for b in range(B):
                bps = ps_big.tile([128, NCH * O], F32, tag="tp")  # share slots w/ tp
                for c in range(NCH):
                    nc.tensor.matmul(
                        bps[:, O * c : O * (c + 1)],
                        lhsT=xT[b][:, c, :],
                        rhs=gt_sb[:, O * b : O * (b + 1)],
                        start=True,
                        stop=True,
                    )
                bv = bps[:].rearrange("p (c o) -> p c o", o=O)
                mx = work.tile([128, NCH], F32, tag="mx")
                nc.vector.reduce_max(mx[:], bv, axis=AX.X)
                bsub = work.tile([128, NCH * O], F32, tag="bsub")
                nc.vector.tensor_sub(
                    bsub[:].rearrange("p (c o) -> p c o", o=O),
                    bv,
                    mx[:].unsqueeze(2).broadcast_to([128, NCH, O]),
                )
                ee = work.tile([128, NCH * O], F32, tag="ee")
                nc.scalar.activation(ee[:], bsub[:], ACTF.Exp)
                den = work.tile([128, NCH], F32, tag="den")
                nc.vector.reduce_sum(
                    den[:], ee[:].rearrange("p (c o) -> p c o", o=O), axis=AX.X
                )
                rcp = work.tile([128, NCH], F32, tag="rcp")
                nc.vector.reciprocal(rcp[:], den[:])
                ct_b = work.tile([128, NCH * O], F32, tag="ct")
                nc.vector.tensor_mul(
                    ct_b[:].rearrange("p (c o) -> p c o", o=O),
                    ee[:].rearrange("p (c o) -> p c o", o=O),
                    rcp[:].unsqueeze(2).broadcast_to([128, NCH, O]),
                )
                cT.append(ct_b)
            # M^T: mt[i, 16b + o] = sum_s c[o, s] x[s, i]
            mt_ps = ps_mt.tile([128, B * O], F32, tag="mt")
            for b in range(B):
                for c in range(NCH):
                    nc.tensor.matmul(
                        mt_ps[:, O * b : O * (b + 1)],
                        lhsT=xnat[b][c // 4][:, c % 4, :],
                        rhs=cT[b][:, O * c : O * (c + 1)],
                        start=(c == 0),
                        stop=(c == NCH - 1),
                    )
            mt_sb = work.tile([128, B * O], F32, tag="mt_sb")
            nc.scalar.copy(mt_sb[:], mt_ps[:])

        # ---- s_r via per-o mini matmuls: sT[d, 16b + o] = s_r[b, o, d] ----
        # One [128, 512] PSUM bank hosts all the small outputs of this iter:
        # cols 0:64 rows 0:32 -> sT; 64:128 row 0 -> n; 128:192 rows 0:32 -> fb;
        # 192:256 -> rep.
        sm = ps_sm.tile([128, 512], F32, tag="sm")
        for o in range(O):
            if r == 0:
                rhs = xsum[:]  # all o share M_0 = Xsum / 16 (scale folded below)
            else:
                rhs = mt_sb[:].rearrange("p (b o) -> p o b", o=O)[:, o, :]
            nc.tensor.matmul(
                sm[0:32, 0:64].rearrange("p (b o) -> p o b", o=O)[:, o, :],
                lhsT=wt[:, 32 * o : 32 * (o + 1)],
                rhs=rhs,
                start=True,
                stop=True,
            )
        st_sb = work.tile([32, 64], F32, tag="st_sb")
        nc.scalar.copy(st_sb[:], sm[0:32, 0:64])
        sq = work.tile([32, 64], F32, tag="sq")
        nc.vector.tensor_mul(sq[:], st_sb[:], st_sb[:])
        nc.tensor.matmul(sm[0:1, 64:128], lhsT=ones32[:], rhs=sq[:], start=True, stop=True)
        # squash factor f = sqrt(n) / (K + n); iter 0 works on 16*s so K = 256,
        # and the 1/16 scales cancel into f = sqrt(n_raw) / (256 + n_raw).
        kconst = 256.0 if r == 0 else 1.0
        dd = work.tile([1, 64], F32, tag="dd")
        nc.vector.tensor_scalar_add(dd[:], sm[0:1, 64:128], kconst)
        rc = work.tile([1, 64], F32, tag="rc")
        nc.vector.reciprocal(rc[:], dd[:])
        # sqrt(n) = exp(0.5 * ln(n)) — keeps ACT in one table set (exp+ln).
        lg = work.tile([1, 64], F32, tag="lg")
        nc.scalar.activation(lg[:], sm[0:1, 64:128], ACTF.Ln)
        sr = work.tile([1, 64], F32, tag="sr")
        nc.scalar.activation(sr[:], lg[:], ACTF.Exp, scale=0.5)
        ff = work.tile([1, 64], F32, tag="ff")
        nc.vector.tensor_mul(ff[:], sr[:], rc[:])
        nc.tensor.matmul(sm[0:32, 128:192], lhsT=ones1[:], rhs=ff[:], start=True, stop=True)
        aT = work.tile([32, 64], F32, tag="aT")
        nc.vector.tensor_mul(aT[:], st_sb[:], sm[0:32, 128:192])

        if r < 2:
            # ArW^T accumulated into gt_ps: spread[od, bo] = a[bo%16, od%32] masked
            nc.tensor.matmul(sm[:, 192:256], lhsT=p4t[:], rhs=aT[:], start=True, stop=True)
            spread = work.tile([128, 4 * B * O], F32, tag="spread")
            nc.vector.tensor_mul(
                spread[:].rearrange("p (c bo) -> p c bo", c=4),
                sm[:, 192:256].unsqueeze(1).broadcast_to([128, 4, B * O]),
                amask[:],
            )
            spv = spread[:].rearrange("p (c bo) -> p c bo", c=4)
            for c in range(4):
                nc.tensor.matmul(
                    sm[:, 256:320],
                    lhsT=w4[:, c, :],
                    rhs=spv[:, c, :],
                    start=(c == 0),
                    stop=(c == 3),
                )
            gt_new = work.tile([128, B * O], F32, tag="gt_sb")
            if r == 0:
                nc.scalar.copy(gt_new[:], sm[:, 256:320])
            else:
                nc.vector.tensor_add(gt_new[:], gt_sb[:], sm[:, 256:320])
            gt_sb = gt_new
        else:
            nc.sync.dma_start(y_d[:].rearrange("b o d -> d (b o)"), aT[:])


_CACHE: dict = {}


def _get_nc():
    if "nc" not in _CACHE:
        nc = bacc.Bacc(None, target_bir_lowering=False)
        with tile.TileContext(nc) as tc:
            _build(tc)
        nc.finalize()
        _CACHE["nc"] = nc
    return _CACHE["nc"]


def kernel(x: np.ndarray, W: np.ndarray) -> np.ndarray:
    from concourse.bass_utils import run_bass_kernel_spmd

    x = np.ascontiguousarray(x, dtype=np.float32)
    W = np.ascontiguousarray(W, dtype=np.float32)
    nc = _get_nc()
    in_maps = [{"x": x[B * k : B * (k + 1)], "W": W} for k in range(N_CORES)]
    res = run_bass_kernel_spmd(nc, in_maps, core_ids=list(range(N_CORES)))
    return np.concatenate([res.results[k]["y"] for k in range(N_CORES)], axis=0)
